# revision 1
# baseline (speedup 1.0000x reference)
"""AdaptiveMultiLoRALinear Trainium2 kernel (8 NeuronCores, data-parallel).

Math (per reference):
  z = x @ W^T + b                               [B,S,O]
  m = sum_e p_e * (x @ A_e @ B_e)               [B,S,O]  (rank-16, 8 experts)
  gamma = min(0.5*||z|| / (||m|| + 1e-6), 1)    per token, norms over O
  out = z + gamma * m

Sharding: data-parallel over the 8192 tokens (1024 per core); W/A/B/b
replicated.  The per-token norms are over the output dim, which every core
holds entirely, so no collectives are needed.

Device kernel per core (bf16 matmuls, f32 accumulation):
  - x [1024,4096] f32 is cast to bf16 (DRAM->DRAM SWDGE cast DMA) and
    transposed into SBUF via the DMA xbar (x^T chunks [128d x 1024t]).
  - z tiles [128t x 512o] accumulate over 32 k-chunks in PSUM; epilogue adds
    bias, squares+reduces for ||z||^2 (ScalarE), spills z to DRAM as bf16.
  - LoRA: U^T = A_st^T x^T (rank 128 = 8 experts x 16), m tiles are single
    matmuls U^T.T @ Bp with p_e folded into Bp host-side.  m is computed
    twice (once for ||m||^2, once fused into the combine) instead of stored.
  - combine: out = z + gamma*m via scalar_tensor_tensor reading m from PSUM.
"""

import sys

sys.path.insert(0, "/opt/trn_rl_repo")

import numpy as np
import ml_dtypes

from concourse import bass, mybir, bacc, tile
from concourse.bass_utils import run_bass_kernel_spmd

BF16 = mybir.dt.bfloat16
F32 = mybir.dt.float32
ALU = mybir.AluOpType
ACTF = mybir.ActivationFunctionType

NCORES = 8
T = 1024          # tokens per core
D = 4096          # input dim
O = 4096          # output dim
ER = 128          # experts * rank
KC = D // 128     # 32 k-chunks
NO = O // 512     # 8 output tiles
MT = T // 128     # 8 token tiles
C_CLAMP = 0.5
EPS = 1e-6

_CACHE = {}


def _build():
    if "nc" in _CACHE:
        return _CACHE["nc"]

    nc = bacc.Bacc(None, target_bir_lowering=False, debug=False)

    x_ext = nc.declare_dram_parameter("x", [T, D], F32, isOutput=False)
    wt_ext = nc.declare_dram_parameter("WT", [NO, 128, KC, 512], BF16, isOutput=False)
    a_ext = nc.declare_dram_parameter("A4", [128, KC, ER], BF16, isOutput=False)
    bp_ext = nc.declare_dram_parameter("Bp", [ER, O], BF16, isOutput=False)
    b_ext = nc.declare_dram_parameter("brep", [128, O], F32, isOutput=False)
    out_ext = nc.declare_dram_parameter("out", [T, O], F32, isOutput=True)

    x_bf = nc.dram_tensor("x_bf", [T, D], BF16)
    z_sp = nc.dram_tensor("z_sp", [MT, 128, O], BF16)

    with tile.TileContext(nc) as tc:
        with (
            tc.tile_pool(name="persist", bufs=1) as pp,
            tc.tile_pool(name="wtp", bufs=2) as wtp,
            tc.tile_pool(name="work", bufs=3) as wk,
            tc.tile_pool(name="psum", bufs=1, space="PSUM") as psp,
        ):
            # ---- persistent loads ----
            bias_sb = pp.tile([128, O], F32)
            nc.sync.dma_start(out=bias_sb[:, :], in_=b_ext[:, :])
            a_sb = pp.tile([128, KC, ER], BF16)
            nc.gpsimd.dma_start(out=a_sb[:, :, :], in_=a_ext[:, :, :])
            bp_sb = pp.tile([ER, O], BF16)
            nc.gpsimd.dma_start(out=bp_sb[:, :], in_=bp_ext[:, :])

            # ---- x cast (f32 -> bf16, DRAM->DRAM) in two row-halves ----
            H = T // 2
            for h in range(2):
                nc.gpsimd.dma_start(
                    out=x_bf[h * H : (h + 1) * H, :], in_=x_ext[h * H : (h + 1) * H, :]
                )

            # ---- x^T via DMA xbar transpose: [128d x (k,1024t)] ----
            xT = pp.tile([128, KC, T], BF16)
            for h in range(2):
                for k in range(KC):
                    nc.sync.dma_start(
                        out=xT[:, k, h * H : (h + 1) * H],
                        in_=x_bf[h * H : (h + 1) * H, k * 128 : (k + 1) * 128],
                        transpose=True,
                    )

            # ---- U^T = A_st^T @ x^T  ([er=128, t=1024], accumulate over k) ----
            uT = pp.tile([ER, T], BF16)
            for h in range(2):
                psu = psp.tile([ER, 512], F32, tag="u", bufs=2)
                for k in range(KC):
                    nc.tensor.matmul(
                        psu[:, :],
                        a_sb[:, k, :],
                        xT[:, k, h * 512 : (h + 1) * 512],
                        start=(k == 0),
                        stop=(k == KC - 1),
                    )
                nc.vector.tensor_copy(uT[:, h * 512 : (h + 1) * 512], psu[:, :])

            # per-(m,n) partial sums of squares
            nz2p = pp.tile([128, MT * NO], F32)
            nm2p = pp.tile([128, MT * NO], F32)

            # ---- pass A: nm2 partials (m = U^T.T @ Bp, square+reduce) ----
            for m in range(MT):
                for n in range(NO):
                    psm = psp.tile([128, 512], F32, tag="mm", bufs=2)
                    nc.tensor.matmul(
                        psm[:, :],
                        uT[:, m * 128 : (m + 1) * 128],
                        bp_sb[:, n * 512 : (n + 1) * 512],
                        start=True,
                        stop=True,
                    )
                    sq = wk.tile([128, 512], F32, tag="sq", bufs=2)
                    nc.scalar.activation(
                        out=sq[:, :],
                        in_=psm[:, :],
                        func=ACTF.Square,
                        accum_out=nm2p[:, m * NO + n : m * NO + n + 1],
                    )

            def finalize(m):
                nz2 = wk.tile([128, 1], F32, tag="s1", bufs=2)
                nc.vector.tensor_reduce(
                    out=nz2[:, :], in_=nz2p[:, m * NO : (m + 1) * NO],
                    axis=mybir.AxisListType.X, op=ALU.add,
                )
                nm2 = wk.tile([128, 1], F32, tag="s2", bufs=2)
                nc.vector.tensor_reduce(
                    out=nm2[:, :], in_=nm2p[:, m * NO : (m + 1) * NO],
                    axis=mybir.AxisListType.X, op=ALU.add,
                )
                nzr = wk.tile([128, 1], F32, tag="s3", bufs=2)
                nc.scalar.sqrt(nzr[:, :], nz2[:, :])
                nmr = wk.tile([128, 1], F32, tag="s4", bufs=2)
                nc.scalar.sqrt(nmr[:, :], nm2[:, :])
                nmre = wk.tile([128, 1], F32, tag="s5", bufs=2)
                nc.vector.tensor_scalar_add(nmre[:, :], nmr[:, :], EPS)
                rmr = wk.tile([128, 1], F32, tag="s6", bufs=2)
                nc.vector.reciprocal(rmr[:, :], nmre[:, :])
                tt = wk.tile([128, 1], F32, tag="s7", bufs=2)
                nc.vector.tensor_tensor(tt[:, :], nzr[:, :], rmr[:, :], op=ALU.mult)
                gam = wk.tile([128, 1], F32, tag="gam", bufs=2)
                nc.vector.tensor_scalar(
                    out=gam[:, :], in0=tt[:, :],
                    scalar1=C_CLAMP, scalar2=1.0, op0=ALU.mult, op1=ALU.min,
                )
                # pass B: recompute m tiles, combine with reloaded z, write out
                for n in range(NO):
                    psb = psp.tile([128, 512], F32, tag="mm", bufs=2)
                    nc.tensor.matmul(
                        psb[:, :],
                        uT[:, m * 128 : (m + 1) * 128],
                        bp_sb[:, n * 512 : (n + 1) * 512],
                        start=True,
                        stop=True,
                    )
                    zre = wk.tile([128, 512], BF16, tag="zre", bufs=3)
                    nc.sync.dma_start(
                        out=zre[:, :], in_=z_sp[m, :, n * 512 : (n + 1) * 512]
                    )
                    ost = wk.tile([128, 512], F32, tag="ost", bufs=3)
                    nc.vector.scalar_tensor_tensor(
                        out=ost[:, :], in0=psb[:, :], scalar=gam[:, 0:1],
                        in1=zre[:, :], op0=ALU.mult, op1=ALU.add,
                    )
                    nc.sync.dma_start(
                        out=out_ext[m * 128 : (m + 1) * 128, n * 512 : (n + 1) * 512],
                        in_=ost[:, :],
                    )

            # ---- phase 1: z tiles ----
            for n in range(NO):
                wt = wtp.tile([128, KC, 512], BF16, tag="wt")
                nc.gpsimd.dma_start(out=wt[:, :, :], in_=wt_ext[n, :, :, :])
                for m in range(MT):
                    ps = psp.tile([128, 512], F32, tag="z", bufs=3)
                    for k in range(KC):
                        nc.tensor.matmul(
                            ps[:, :],
                            xT[:, k, m * 128 : (m + 1) * 128],
                            wt[:, k, :],
                            start=(k == 0),
                            stop=(k == KC - 1),
                        )
                    zt = wk.tile([128, 512], BF16, tag="zt", bufs=3)
                    nc.vector.tensor_tensor(
                        out=zt[:, :], in0=ps[:, :],
                        in1=bias_sb[:, n * 512 : (n + 1) * 512], op=ALU.add,
                    )
                    sq = wk.tile([128, 512], F32, tag="sq", bufs=2)
                    nc.scalar.activation(
                        out=sq[:, :], in_=zt[:, :], func=ACTF.Square,
                        accum_out=nz2p[:, m * NO + n : m * NO + n + 1],
                    )
                    nc.sync.dma_start(
                        out=z_sp[m, :, n * 512 : (n + 1) * 512], in_=zt[:, :]
                    )
                    if n == NO - 1:
                        finalize(m)

    nc.compile()
    _CACHE["nc"] = nc
    return nc


def _prep(x, W, b, A, B, p_scores):
    x = np.ascontiguousarray(np.asarray(x, dtype=np.float32)).reshape(-1, D)
    W = np.asarray(W, dtype=np.float32)
    b = np.asarray(b, dtype=np.float32)
    A = np.asarray(A, dtype=np.float32)
    B = np.asarray(B, dtype=np.float32)
    p_scores = np.asarray(p_scores, dtype=np.float32)

    bf = ml_dtypes.bfloat16
    # W^T tiled [n, p, k, o]: WT_t[n,p,k,o] = W[n*512+o, k*128+p]
    wt_t = np.ascontiguousarray(
        W.T.reshape(KC, 128, NO, 512).transpose(2, 1, 0, 3)
    ).astype(bf)
    # A stacked [p, k, er]: A4[p,k,e*16+r] = A[e, k*128+p, r]
    a_st = A.transpose(1, 0, 2).reshape(D, ER)          # [d, er]
    a4 = np.ascontiguousarray(a_st.reshape(KC, 128, ER).transpose(1, 0, 2)).astype(bf)
    bp = np.ascontiguousarray(
        (p_scores[:, None, None] * B).reshape(ER, O)
    ).astype(bf)
    brep = np.ascontiguousarray(np.broadcast_to(b, (128, O))).astype(np.float32)

    in_maps = []
    for i in range(NCORES):
        in_maps.append(
            {
                "x": np.ascontiguousarray(x[i * T : (i + 1) * T]),
                "WT": wt_t,
                "A4": a4,
                "Bp": bp,
                "brep": brep,
            }
        )
    return in_maps


def run(inputs, trace=False):
    nc = _build()
    in_maps = _prep(**inputs)
    res = run_bass_kernel_spmd(nc, in_maps, list(range(NCORES)), trace=trace)
    out = np.concatenate([r["out"] for r in res.results], axis=0)
    return out.reshape(4, 2048, 4096).astype(np.float32), res


def kernel(**inputs):
    out, _ = run(inputs, trace=False)
    return out
